# revision 37
# baseline (speedup 1.0000x reference)
"""Trainium2 Bass kernel for nn_MultiHeadLiftLayer (GNN edge-signal lift).

Computes, for each edge e with endpoints (src, tgt):
    out[e, k] = relu( x[src] . a_src[k]  +  x[tgt] . a_tgt[k] ),  k = 0..3

Strategy (edge-parallel across 8 NeuronCores):
  - Edges are bucketed by (src-half, tgt-half) class; each class is
    sorted by src and split EVENLY across the 8 cores, so every core
    bucket holds ~global_class/8 edges (tight balance lets the bucket
    capacity sit at 28 chunks) and each core's src gathers touch only a
    contiguous ~1/8 span of the table (HBM row locality). Parts are
    re-sorted by tgt so the tgt gathers walk the table in order too.
  - Per core, each edge endpoint's x row (64 fp16 values padded to 128 =
    256B, the SWDGE dma_gather minimum element) is fetched with batched
    dma_gather instructions in TRANSPOSE mode: one instruction gathers
    896 rows and lands them feature-major [128 feats, 896 edges] in
    SBUF, ready to be the PE matmul moving operand.
  - The per-edge projection + add comes free on the PE: psum[4, e] is
    accumulated over two matmuls (a_src.T @ Xs then a_tgt.T @ Xt with
    start/stop accumulation), then ACT applies relu and the [4, e]
    K-major result is DMA'd out. The host transposes back to (E, 4).
  - dma_gather indices are int16 (max 32767) but N=50000, so x is staged
    as TWO half-tables of 26624 rows and edges are bucketed host-side by
    (src-half, tgt-half) into 4 buckets; each bucket does its src gather
    from table hs and tgt gather from table ht with half-local indices.
    Bucket slots are padded to a fixed capacity with index 0 (valid row,
    results dropped on host) so the program stays static. In the
    (pathological) case a bucket overflows its capacity, the same
    program is simply run again on the leftover edges.
"""

import numpy as np

import concourse.bacc as bacc
import concourse.mybir as mybir
import concourse.tile as tile
from concourse.bass_utils import run_bass_kernel_spmd
from concourse.instruction_name_ordered_set import InstructionNameOrderedSet

# ---- problem constants (hardcoded per contract) ----
N_NODES = 50000
N_EDGES = 800000
F_IN = 64
K = 4
CORES = 8

SPLIT = 25000                # node id threshold between the two halves
NH = 26624                   # rows per half-table (>= SPLIT)
E_C = N_EDGES // CORES       # 100000 edges per core
CHUNK = 896                  # max num_idxs per transpose dma_gather (ucode
                             # idx-staging limit; 1024 crashes)
NCHUNK = 28                  # chunks per bucket-side
CAP = CHUNK * NCHUNK         # 25088; each (src-half, tgt-half) class is
                             # split EVENLY across cores, so a core bucket
                             # holds ~global_class/8 (~25000 +- 35)
ICOLS = CAP // 16            # idx columns per bucket-side (wrapped layout)
MM = 512                     # psum sub-chunk (PSUM bank = 512 f32)

F32 = mybir.dt.float32
F16 = mybir.dt.float16
I16 = mybir.dt.int16

_PROGRAM_CACHE = {}


def _build_program():
    nc = bacc.Bacc("TRN2", num_swdge_queues=4)

    tb = [
        nc.dram_tensor(f"tb{h}", [NH, 128], F16, kind="ExternalInput")
        for h in (0, 1)
    ]
    a_in = nc.dram_tensor("a_in", [64, 8], F16, kind="ExternalInput")
    # 8 bucket-sides packed: [(b0,src),(b0,tgt),(b1,src),...] each ICOLS wide
    idx_in = nc.dram_tensor("idx_in", [128, 8 * ICOLS], I16,
                            kind="ExternalInput")
    out_d = nc.dram_tensor("out", [4, 4 * CAP], F32, kind="ExternalOutput")

    with tile.TileContext(nc) as tc:
        with (
            tc.tile_pool(name="const", bufs=1) as cpool,
            tc.tile_pool(name="gath", bufs=4) as gpool,
            tc.tile_pool(name="ps", bufs=8, space="PSUM") as ppool,
            tc.tile_pool(name="rel", bufs=4) as rpool,
        ):
            # stage PE weights through a DVE copy so matmul deps ride the
            # single-sync-wait LDWEIGHTS path cleanly
            a_raw = cpool.tile([64, 8], F16)
            nc.sync.dma_start(out=a_raw[:], in_=a_in[:])
            a_sb = cpool.tile([64, 8], F16)
            nc.vector.tensor_copy(out=a_sb[:], in_=a_raw[:])

            idx_sbs = []
            for i in range(8):
                t = cpool.tile([128, ICOLS], I16, tag=f"idx{i}")
                nc.sync.dma_start(
                    out=t[:], in_=idx_in[:, i * ICOLS:(i + 1) * ICOLS])
                idx_sbs.append(t)

            all_g = []

            def emit_gather(b, side, h, ci, tag):
                off = ci * CHUNK
                g = gpool.tile([128, CHUNK], F16, tag=tag,
                               name=f"g_{b}_{side}_{ci}")
                isb = idx_sbs[2 * b + side]
                c0 = off // 16
                gi = nc.gpsimd.dma_gather(
                    out_ap=g[:].rearrange("p (o m) -> p o m", o=1),
                    in_ap=tb[h][:, :],
                    idxs_ap=isb[:, c0:c0 + CHUNK // 16],
                    num_idxs=CHUNK,
                    num_idxs_reg=CHUNK,
                    elem_size=128,
                    transpose=True,
                    queue_num=len(all_g) % 4,
                )
                # pin engine order so lane/queue assignment stays
                # deterministic under the tile scheduler
                if all_g:
                    ns = InstructionNameOrderedSet()
                    ns.add(all_g[-1].ins.name)
                    gi.ins.add_nosync_dependencies_from(ns)
                # one entry in flight per queue: wait for the DMA
                # completion of this queue's previous gather (NOT its
                # consumer). Chunk-PAIR emission order (s,s,t,t) gives
                # the slow random-side gathers a 4-position queue
                # revisit so the previous drain is done by reissue.
                if len(all_g) >= 4:
                    tile.add_dep_helper(
                        gi.ins, all_g[-4].ins,
                        reason="swdge queue spacing")
                all_g.append(gi)
                return g

            for b in range(4):
                hs, ht = b >> 1, b & 1
                for cp in range(0, NCHUNK, 2):
                    gs = [emit_gather(b, 0, hs, cp + i, f"gs{i}")
                          for i in (0, 1)]
                    gt = [emit_gather(b, 1, ht, cp + i, f"gt{i}")
                          for i in (0, 1)]
                    for i in (0, 1):
                        ci = cp + i
                        off = ci * CHUNK
                        r = rpool.tile([4, CHUNK], F32)
                        for mi in range(2):
                            s0 = mi * MM
                            mw = min(MM, CHUNK - s0)
                            ps = ppool.tile([4, MM], F32)
                            nc.tensor.matmul(
                                out=ps[:, :mw],
                                lhsT=a_sb[:, 0:4],
                                rhs=gs[i][0:64, s0:s0 + mw],
                                start=True,
                                stop=False,
                            )
                            nc.tensor.matmul(
                                out=ps[:, :mw],
                                lhsT=a_sb[:, 4:8],
                                rhs=gt[i][0:64, s0:s0 + mw],
                                start=False,
                                stop=True,
                            )
                            nc.scalar.activation(
                                out=r[:, s0:s0 + mw], in_=ps[:, :mw],
                                func=mybir.ActivationFunctionType.Relu,
                            )
                        o0 = b * CAP + off
                        nc.sync.dma_start(
                            out=out_d[:, o0:o0 + CHUNK], in_=r[:],
                        )

    # tile assigns each gather a DMASW completion-sem lane (mod 8) in
    # SCHEDULED order; each lane must stay on one SWDGE queue, so derive
    # queue_num from the lane. Gather order is pinned by nosync deps and
    # each gather carries an explicit sem dep on the DMA completion of
    # its queue predecessor (j -> j-4), so exactly one entry is in flight
    # per queue regardless of tile depth (deeper queuing corrupts).
    from concourse.tile_sem_assignment import PROC_NAME_TO_IDX
    lane_of = {PROC_NAME_TO_IDX[f"DMASW{i}"]: i for i in range(8)}
    for blk in nc.main_func.blocks:
        for inst in blk.instructions:
            if isinstance(inst, mybir.InstDMAGatherAnt):
                lane = lane_of.get(inst.bass_scheduled_proc)
                if lane is not None:
                    inst.queue_num = lane % 4

    nc.compile()
    return nc


def get_program():
    if "nc" not in _PROGRAM_CACHE:
        _PROGRAM_CACHE["nc"] = _build_program()
    return _PROGRAM_CACHE["nc"]


def _wrap_idx(lst):
    """Wrap an index list (len CAP) for SWDGE: idx j -> [16g + j%16, j//16]
    replicated across the 8 GPSIMD cores (g = 0..7)."""
    w = lst.reshape(ICOLS, 16).T.astype(np.int16)   # [16, ICOLS]
    return np.tile(w, (8, 1))                       # [128, ICOLS]


def make_tables(x, att):
    x = np.asarray(x, dtype=np.float32)
    att = np.asarray(att, dtype=np.float32)
    xt = np.zeros((SPLIT + NH, 128), dtype=np.float16)
    xt[:N_NODES, :F_IN] = x.astype(np.float16)
    a = np.empty((F_IN, 8), dtype=np.float16)
    a[:, :K] = att[:, :F_IN].T.astype(np.float16)
    a[:, K:] = att[:, F_IN:].T.astype(np.float16)
    return xt[:NH], xt[SPLIT:SPLIT + NH], a


def prepare_passes(x, edge_index, att):
    """Host marshaling: shard edges across cores BY CLASS, build in_maps.

    Each (src-half, tgt-half) class is sorted by src (HBM locality for the
    src gathers) and split evenly across the 8 cores, so every per-core
    bucket holds ~global_class/8 edges (sigma ~35) and fits CAP with wide
    margin. slot_maps[c] lists (bucket, global_edge_ids) per output slot.
    Normally a single pass; more only if a class exceeds 8*CAP.
    """
    tb0, tb1, a = make_tables(x, att)
    ei = np.asarray(edge_index).astype(np.int64)
    s, t = ei[0], ei[1]
    bid = (s >= SPLIT) * 2 + (t >= SPLIT)

    parts = {}
    max_len = 0
    for b in range(4):
        ids = np.where(bid == b)[0]
        ids = ids[np.argsort(s[ids], kind="stable")]
        bounds = np.linspace(0, len(ids), CORES + 1).astype(np.int64)
        for c in range(CORES):
            seg = ids[bounds[c]:bounds[c + 1]]
            # Z-order (Morton) sort over (src, tgt): one linear order
            # cannot be ascending in both endpoints, but the Z-curve
            # clusters consecutive slots in BOTH coordinates at every
            # scale, so src AND tgt gathers each get DRAM row locality
            if len(seg):
                sr = s[seg] - s[seg].min()
                tr = t[seg] - (b & 1) * SPLIT
                key = np.zeros(len(seg), np.int64)
                for i in range(15):
                    key |= ((sr >> i) & 1) << (2 * i + 1)
                    key |= ((tr >> i) & 1) << (2 * i)
                seg = seg[np.argsort(key, kind="stable")]
            parts[(c, b)] = seg
            max_len = max(max_len, len(seg))

    n_pass = max(1, -(-max_len // CAP))
    passes = []
    for p in range(n_pass):
        in_maps = []
        slot_maps = []
        for c in range(CORES):
            idx_arr = np.zeros((128, 8 * ICOLS), dtype=np.int16)
            slots = []  # (bucket, global edge ids) for this pass
            for b in range(4):
                eids = parts[(c, b)][p * CAP:(p + 1) * CAP]
                sl = np.zeros(CAP, dtype=np.int64)
                tl = np.zeros(CAP, dtype=np.int64)
                sl[:len(eids)] = s[eids] - (b >> 1) * SPLIT
                tl[:len(eids)] = t[eids] - (b & 1) * SPLIT
                idx_arr[:, (2 * b) * ICOLS:(2 * b + 1) * ICOLS] = \
                    _wrap_idx(sl)
                idx_arr[:, (2 * b + 1) * ICOLS:(2 * b + 2) * ICOLS] = \
                    _wrap_idx(tl)
                slots.append((b, eids))
            in_maps.append({
                "tb0": tb0, "tb1": tb1, "a_in": a, "idx_in": idx_arr,
            })
            slot_maps.append(slots)
        passes.append((in_maps, slot_maps))
    return passes


TRACE = False           # test harness hook: set True to request NTFF trace
LAST_RESULTS = []       # test harness hook: BassSpmdResult of each pass


def kernel(x, edge_index, att):
    nc = get_program()
    out = np.empty((N_EDGES, K), dtype=np.float32)
    LAST_RESULTS.clear()
    for in_maps, slot_maps in prepare_passes(x, edge_index, att):
        res = run_bass_kernel_spmd(
            nc, in_maps, core_ids=list(range(CORES)), trace=TRACE
        )
        LAST_RESULTS.append(res)
        for c in range(CORES):
            o = np.asarray(res.results[c]["out"])  # [4, 4*CAP]
            for b, eids in slot_maps[c]:
                if len(eids):
                    out[eids] = o[:, b * CAP:b * CAP + len(eids)].T
    return out
